# revision 8
# baseline (speedup 1.0000x reference)
"""GCN conv kernel for TRN2, build 4: capped cells + pooled overflow chunks.

Math: out = segment_sum(edge_weight * X[edge_col], edge_row) @ W + bias.

vs build 2e: each (block, cchunk) cell gets a fixed 4-chunk (512-edge)
budget; edges beyond 512 spill into per-(tile, cchunk) overflow chunks whose
one-hot B is tile-relative (256 wide). All 4 blocks of a tile accumulate in
ONE psum bank (64-col windows), so overflow matmuls can target the whole
tile. Cuts gather slots 501,760 -> ~427k (-14.9%); the SWDGE descriptor-gen
loop (~2ns/idx) is the hard bottleneck, so slots ~= time.
"""
import sys
sys.path.insert(0, "/opt/trn_rl_repo")

import numpy as np
from dataclasses import dataclass, field

import concourse.bass as bass
import concourse.bacc as bacc
from concourse import mybir
from concourse import library_config

F32 = mybir.dt.float32
F16 = mybir.dt.float16
I16 = mybir.dt.int16


@dataclass
class Cfg:
    n_nodes: int = 100000
    n_cores: int = 8
    S: int = 64                 # rows per block
    TB: int = 4                 # blocks per tile
    RC: int = 4                 # regular chunks per cell (cap = RC*128 edges)
    chunk_rows: int = 25088     # table rows per int16 chunk (<= 32767)
    D: int = 128
    nq: int = 4
    OB: object = None           # [NT,4] overflow chunk budgets (data-driven)

    @property
    def n_pad(self):
        return 4 * self.chunk_rows

    @property
    def nb_core(self):
        return self.n_pad // self.S // self.n_cores

    @property
    def rows_core(self):
        return self.nb_core * self.S

    @property
    def nt(self):
        assert self.nb_core % self.TB == 0
        return self.nb_core // self.TB

    @property
    def obt(self):              # per-tile total overflow chunks
        return self.OB.sum(axis=1)

    @property
    def obt_max(self):
        return int(self.obt.max())

    @property
    def msl(self):              # meta cols per tile slot
        return 2 * self.TB * 16 // 2 * 2 + 2 * self.obt_max  # 128 + 2*OBT_MAX

    @property
    def gch(self):              # chunks per tile (incl overflow)
        return 16 * self.TB // self.TB * self.TB + 0  # placeholder (unused)

    @property
    def gch_max(self):
        return 4 * 16 + self.obt_max  # 64 regular + overflow

    @property
    def tic_max(self):          # idx cols per tile slot
        return self.gch_max * 128 // 16


def compute_budgets(edge_row, edge_col, cfg: Cfg):
    c = cfg
    row = np.asarray(edge_row, np.int64)
    col = np.asarray(edge_col, np.int64)
    NB, NT, TB = c.nb_core, c.nt, c.TB
    core = row // c.rows_core
    block = (row % c.rows_core) // c.S
    cc = col // c.chunk_rows
    cell = (core * NB + block) * 4 + cc
    cnt = np.bincount(cell, minlength=c.n_cores * NB * 4).reshape(c.n_cores, NB, 4)
    ovf = np.maximum(cnt - c.RC * 128, 0)
    ovf_tc = ovf.reshape(c.n_cores, NT, TB, 4).sum(axis=2)
    return np.ceil(ovf_tc / 128).astype(np.int64).max(axis=0)   # [NT, 4]


def host_prep(inputs, edge_row, edge_col, edge_weight, cfg: Cfg):
    c = cfg
    OB = c.OB
    NB, NT, TB, S = c.nb_core, c.nt, c.TB, c.S
    RC = c.RC
    OBT_MAX, MSL, GCH_MAX, TIC_MAX = c.obt_max, c.msl, c.gch_max, c.tic_max

    table = np.zeros((c.n_pad, c.D), np.float16)
    table[: c.n_nodes] = inputs.astype(np.float16)

    row = np.asarray(edge_row, np.int64)
    col = np.asarray(edge_col, np.int64)
    w = np.asarray(edge_weight, np.float32)

    core = row // c.rows_core
    block = (row % c.rows_core) // c.S
    rowlocal = (row % c.S).astype(np.int64)
    cc = col // c.chunk_rows
    idx16 = (col % c.chunk_rows).astype(np.int16)

    cell = (core * NB + block) * 4 + cc
    order = np.lexsort((col, cell))
    cell_s = cell[order]
    idx16_s = idx16[order]
    rl_s = rowlocal[order]
    w_s = w[order]

    counts = np.bincount(cell_s, minlength=c.n_cores * NB * 4)
    starts = np.zeros_like(counts)
    starts[1:] = np.cumsum(counts)[:-1]
    offset = np.arange(len(cell_s)) - starts[cell_s]

    core_s = cell_s // (NB * 4)
    rem = cell_s % (NB * 4)
    block_s = rem // 4
    cc_s = rem % 4
    t_s = block_s // TB
    b_s = block_s % TB

    # per-tile chunk geometry (same across cores)
    goff = np.zeros((NT, 4), np.int64)          # chunk offset of call (t,cc)
    for t in range(NT):
        acc = 0
        for g in range(4):
            goff[t, g] = acc
            acc += 4 * RC + OB[t, g]            # 16 regular + OB overflow
    GCH = goff[:, 3] + 4 * RC + OB[:, 3]        # chunks per tile
    OBOFF = np.cumsum(np.pad(OB, ((0, 0), (1, 0)))[:, :4], axis=1)  # [NT,4]

    reg = offset < RC * 128
    q_r = offset // 128
    p_r = offset % 128

    # overflow position within (core, t, cc): cumulative over blocks
    oo = np.where(reg, 0, offset - RC * 128)
    ovf_cnt = np.maximum(
        np.bincount(cell_s[~reg] // 1,
                    minlength=c.n_cores * NB * 4), 0)  # per-cell overflow count
    ovf_cnt = np.minimum(counts - np.minimum(counts, RC * 128), 1 << 30)
    ovf_cell = ovf_cnt.reshape(c.n_cores, NB, 4)
    # start of each cell's overflow run within its (core,t,cc) pool
    ovf_start = np.zeros_like(ovf_cell)
    ovf_tc_view = ovf_cell.reshape(c.n_cores, NT, TB, 4)
    ovf_start_view = ovf_start.reshape(c.n_cores, NT, TB, 4)
    np.cumsum(ovf_tc_view[:, :, :-1, :], axis=2, out=ovf_start_view[:, :, 1:, :])
    p_o = ovf_start.reshape(-1)[cell_s] + oo    # position in (t,cc) ovf pool
    assert (p_o[~reg] < OB[t_s[~reg], cc_s[~reg]] * 128).all(), "ovf budget"

    chunk_in_tile = np.where(
        reg, goff[t_s, cc_s] + b_s * (4 * RC) // 1 + q_r,
        goff[t_s, cc_s] + 4 * RC + p_o // 128)
    # fix regular: b_s*4RC is wrong (per-block 4 chunks within the call)
    chunk_in_tile = np.where(
        reg, goff[t_s, cc_s] + b_s * RC + q_r,
        goff[t_s, cc_s] + 4 * RC + p_o // 128)
    part = np.where(reg, p_r, p_o % 128)

    GBASE = np.zeros(NT + 1, np.int64)
    GBASE[1:] = np.cumsum(GCH)
    nslots = int(GBASE[NT]) * 128
    gslot = (GBASE[t_s] + chunk_in_tile) * 128 + part

    # meta columns
    mcol_rl = np.where(
        reg, t_s * MSL + b_s * (4 * RC) + cc_s * RC + q_r,
        t_s * MSL + 2 * 4 * RC * TB // 2 + OBOFF[t_s, cc_s] + p_o // 128)
    # regular rl plane [0,64), w plane [64,128); ovf rl at 128+, w at 128+OBT_MAX+
    REG_W_OFF = 4 * RC * TB            # 64
    OVF_BASE = 2 * REG_W_OFF           # 128
    mcol_rl = np.where(
        reg, t_s * MSL + b_s * (4 * RC) + cc_s * RC + q_r,
        t_s * MSL + OVF_BASE + OBOFF[t_s, cc_s] + p_o // 128)
    mcol_w = np.where(
        reg, mcol_rl + REG_W_OFF, mcol_rl + OBT_MAX)
    rl_val = np.where(reg, rl_s, b_s * S + rl_s).astype(np.float32)

    in_maps = []
    for k in range(c.n_cores):
        sel = core_s == k
        idx_flat = np.zeros(nslots, np.int16)
        idx_flat[gslot[sel]] = idx16_s[sel]
        rl_flat = np.full((128, NT * MSL), 9999.0, np.float32)
        w_flat = np.zeros((128, NT * MSL), np.float32)
        rl_flat[part[sel], mcol_rl[sel]] = rl_val[sel]
        w_flat[part[sel], mcol_w[sel]] = w_s[sel]

        # idx wrap per call into fixed [128, NT*TIC_MAX] dram layout
        idx_dram = np.zeros((128, NT * TIC_MAX), np.int16)
        for t in range(NT):
            icol = 0
            for g in range(4):
                cap = (4 * RC + OB[t, g]) * 128
                sl = idx_flat[(GBASE[t] + goff[t, g]) * 128:
                              (GBASE[t] + goff[t, g]) * 128 + cap]
                a = sl.reshape(cap // 16, 16).T           # [16, cap/16]
                a = np.tile(a, (8, 1))                    # [128, cap/16]
                idx_dram[:, t * TIC_MAX + icol:
                         t * TIC_MAX + icol + cap // 16] = a
                icol += cap // 16

        meta = np.empty((128, NT * MSL), np.float16)
        meta[:] = rl_flat.astype(np.float16)
        wm = w_flat.astype(np.float16)
        nz = wm != 0
        meta[nz] = wm[nz]
        # the line above is wrong for overlapping cols; build properly:
        meta = rl_flat.astype(np.float16)
        for t in range(NT):
            base = t * MSL
            meta[:, base + REG_W_OFF:base + OVF_BASE] = \
                wm[:, base + REG_W_OFF:base + OVF_BASE]
            meta[:, base + OVF_BASE + OBT_MAX:base + OVF_BASE + 2 * OBT_MAX] = \
                wm[:, base + OVF_BASE + OBT_MAX:base + OVF_BASE + 2 * OBT_MAX]

        in_maps.append({
            "table": table,
            "idx_in": np.ascontiguousarray(idx_dram),
            "meta_in": np.ascontiguousarray(meta),
        })
    return in_maps


def add_consts(in_maps, weight, bias, cfg):
    c = cfg
    wb = np.asarray(weight, np.float32)
    bb = np.tile(np.asarray(bias, np.float32)[None, :], (128, 1))
    iota = np.tile(np.arange(c.S, dtype=np.float16)[None, :], (128, 1))
    iota256 = np.tile(np.arange(4 * c.S, dtype=np.float16)[None, :], (128, 1))
    zeros = np.zeros((128, 4 * c.S), np.float16)
    for m in in_maps:
        m["w_in"] = wb
        m["bias_in"] = np.ascontiguousarray(bb)
        m["iota_in"] = np.ascontiguousarray(iota)
        m["iota256_in"] = np.ascontiguousarray(iota256)
        m["zero_in"] = np.ascontiguousarray(zeros)


def build(cfg: Cfg):
    c = cfg
    OB = c.OB
    NT, TB, S, D = c.nt, c.TB, c.S, c.D
    RC = c.RC
    NB = c.nb_core
    NF = c.rows_core // 128
    CH = c.chunk_rows
    FPT = TB * S // 128              # final row-tiles per tile (=2)
    OBT_MAX, MSL, GCH_MAX, TIC_MAX = c.obt_max, c.msl, c.gch_max, c.tic_max
    OBT = [int(OB[t].sum()) for t in range(NT)]
    REG_W_OFF = 4 * RC * TB          # 64
    OVF_BASE = 2 * REG_W_OFF         # 128
    goff = []
    for t in range(NT):
        acc, row_ = 0, []
        for g in range(4):
            row_.append(acc)
            acc += 4 * RC + int(OB[t, g])
        goff.append(row_)
    oboff = [[int(OB[t, :g].sum()) for g in range(4)] for t in range(NT)]
    BSR = 4 * RC * S                 # 1024: regular B cols per block

    nc = bacc.Bacc("TRN2", target_bir_lowering=False, debug=False,
                   num_devices=c.n_cores, num_swdge_queues=c.nq)
    table = nc.dram_tensor("table", [c.n_pad, D], F16, kind="ExternalInput")
    w_in = nc.dram_tensor("w_in", [D, D], F32, kind="ExternalInput")
    bias_in = nc.dram_tensor("bias_in", [128, D], F32, kind="ExternalInput")
    idx_in = nc.dram_tensor("idx_in", [128, NT * TIC_MAX], I16,
                            kind="ExternalInput")
    meta_in = nc.dram_tensor("meta_in", [128, NT * MSL], F16,
                             kind="ExternalInput")
    iota_in = nc.dram_tensor("iota_in", [128, S], F16, kind="ExternalInput")
    iota256_in = nc.dram_tensor("iota256_in", [128, 4 * S], F16,
                                kind="ExternalInput")
    zero_in = nc.dram_tensor("zero_in", [128, 4 * S], F16, kind="ExternalInput")
    out = nc.dram_tensor("out", [c.rows_core, D], F32, kind="ExternalOutput")

    from contextlib import ExitStack
    with ExitStack() as _es:
        def sb(name, shape, dt):
            return _es.enter_context(nc.sbuf_tensor(name, shape, dt))
        def ps(name):
            return _es.enter_context(nc.psum_tensor(name, [128, 512], F32))
        def sem(name):
            return _es.enter_context(nc.semaphore(name))
        g_sb = sb("g_sb", [128, 4, GCH_MAX, 128], F16)
        idx_sb = sb("idx_sb", [128, 4, TIC_MAX], I16)
        meta_sb = sb("meta_sb", [128, 4, MSL], F16)
        b_sb = sb("b_sb", [128, 8, BSR], F16)
        bo_sb = sb("bo_sb", [128, 4, OBT_MAX * 4 * S], F16)
        agg_sb = sb("agg_sb", [128, NB * S], F32)
        w_sb = sb("w_sb", [128, D], F32)
        bias_sb = sb("bias_sb", [128, D], F32)
        iota_f = sb("iota_f", [128, S], F16)
        iota256_f = sb("iota256_f", [128, 4 * S], F16)
        zeros_f = sb("zeros_f", [128, 4 * S], F16)
        ostage = sb("ostage", [128, 2, D], F32)
        psb = [ps("ps0"), ps("ps1"), ps("ps2"), ps("ps3")]
        pfin = [ps("pf0"), ps("pf1")]
        const_io = sem("const_io")
        idx_s = [sem(f"idx_s{i}") for i in range(4)]
        meta_s = [sem(f"meta_s{i}") for i in range(4)]
        g_s = [[sem(f"g_s{q}_{s}") for s in range(4)] for q in range(4)]
        ost_s = [sem("ost_s0"), sem("ost_s1")]
        dve_prog = sem("dve_prog")           # 1 per regular B block
        dve_ovf = sem("dve_ovf")             # 1 per tile's overflow B
        pe_tiles, act_prog = sem("pe_tiles"), sem("act_prog")
        bb = sem("bb")
        bbo = sem("bbo")
        pe_fin, dve_fin = sem("pe_fin"), sem("dve_fin")
        block = _es.enter_context(nc.Block())

        @block.sync
        def _(sync: bass.BassEngine):
            sync.dma_start(w_sb[:, :], w_in[:, :]).then_inc(const_io, 16)
            sync.dma_start(bias_sb[:, :], bias_in[:, :]).then_inc(const_io, 16)
            sync.dma_start(iota_f[:, :], iota_in[:, :]).then_inc(const_io, 16)
            sync.dma_start(iota256_f[:, :], iota256_in[:, :]).then_inc(const_io, 16)
            sync.dma_start(zeros_f[:, :], zero_in[:, :]).then_inc(const_io, 16)
            for t in range(NT):
                if t >= 4:
                    for q in range(4):
                        sync.wait_ge(g_s[q][t % 4], 16 * ((t - 4) // 4 + 1))
                sync.dma_start(
                    idx_sb[:, t % 4, :],
                    idx_in[:, t * TIC_MAX:(t + 1) * TIC_MAX],
                ).then_inc(idx_s[t % 4], 16)
                if t >= 4:
                    sync.wait_ge(dve_ovf, t - 3)
                sync.dma_start(
                    meta_sb[:, t % 4, :],
                    meta_in[:, t * MSL:(t + 1) * MSL],
                ).then_inc(meta_s[t % 4], 16)

        @block.gpsimd
        def _(gp: bass.BassGpSimd):
            gp.load_library(library_config.mlp)
            for t in range(NT):
                gp.wait_ge(idx_s[t % 4], 16 * (t // 4 + 1))
                if t >= 4:
                    gp.wait_ge(pe_tiles, t - 3)
                icol = 0
                for g in range(4):
                    cap = (4 * RC + int(OB[t, g])) * 128
                    gp.dma_gather(
                        g_sb[:, t % 4, goff[t][g]:goff[t][g] + cap // 128, :],
                        table[g * CH:(g + 1) * CH, :],
                        idx_sb[:, t % 4, icol:icol + cap // 16],
                        cap, cap, 128, single_packet=False, queue_num=g,
                    ).then_inc(g_s[g][t % 4], 16)
                    icol += cap // 16

        @block.vector
        def _(dve: bass.BassEngine):
            dve.wait_ge(const_io, 80)

            def final_tt(f):
                dve.wait_ge(pe_fin, f + 1)
                if f >= 2:
                    dve.wait_ge(ost_s[f % 2], 16 * (f // 2))
                dve.tensor_tensor(
                    ostage[:, f % 2, :], pfin[f % 2][:, :D], bias_sb[:, :],
                    mybir.AluOpType.add,
                ).then_inc(dve_fin, 1)

            mp = 4 * MSL
            for t in range(NT):
                dve.wait_ge(meta_s[t % 4], 16 * (t // 4 + 1))
                for b in range(TB):
                    gb = t * TB + b
                    if gb >= 8:
                        dve.wait_ge(pe_tiles, t - 1)
                    moff = (t % 4) * MSL + b * 4 * RC
                    rl_ap = bass.AP(meta_sb, moff,
                                    [[mp, 128], [1, 4 * RC], [0, S]])
                    w_ap = bass.AP(meta_sb, moff + REG_W_OFF,
                                   [[mp, 128], [1, 4 * RC], [0, S]])
                    io_ap = bass.AP(iota_f, 0, [[S, 128], [0, 4 * RC], [1, S]])
                    b3 = bass.AP(b_sb, (gb % 8) * BSR,
                                 [[8 * BSR, 128], [S, 4 * RC], [1, S]])
                    dve.tensor_tensor(b3, io_ap, rl_ap,
                                      mybir.AluOpType.is_equal).then_inc(bb, 1)
                    dve.tensor_tensor(b3, b3, w_ap,
                                      mybir.AluOpType.mult)._wait_ge(
                        bb, gb + 1).then_inc(dve_prog, 1)
                # overflow B for the tile (256-wide rowlocal)
                nob = OBT[t]
                if nob > 0:
                    if t >= 4:
                        dve.wait_ge(pe_tiles, t - 3)
                    moff = (t % 4) * MSL + OVF_BASE
                    rlo = bass.AP(meta_sb, moff,
                                  [[mp, 128], [1, nob], [0, 4 * S]])
                    wo = bass.AP(meta_sb, moff + OBT_MAX,
                                 [[mp, 128], [1, nob], [0, 4 * S]])
                    ioo = bass.AP(iota256_f, 0, [[4 * S, 128], [0, nob], [1, 4 * S]])
                    b3o = bass.AP(bo_sb, (t % 4) * OBT_MAX * 4 * S,
                                  [[4 * OBT_MAX * 4 * S, 128], [4 * S, nob], [1, 4 * S]])
                    dve.tensor_tensor(b3o, ioo, rlo,
                                      mybir.AluOpType.is_equal).then_inc(bbo, 1)
                    dve.tensor_tensor(b3o, b3o, wo,
                                      mybir.AluOpType.mult)._wait_ge(
                        bbo, t + 1).then_inc(dve_ovf, 1)
                else:
                    dve.sem_inc(dve_ovf, 1)
                if t >= 1:
                    for f in range(FPT * (t - 1), FPT * t):
                        final_tt(f)
            for f in range(FPT * (NT - 1), NF):
                final_tt(f)

        @block.tensor
        def _(pe: bass.BassEngine):
            pe.wait_ge(const_io, 80)

            def final_mm(f):
                pe.wait_ge(act_prog, f // 2 + 1)
                if f >= 2:
                    pe.wait_ge(dve_fin, f - 1)
                pe.matmul(
                    pfin[f % 2][:, :D],
                    agg_sb[:, f * 128:(f + 1) * 128],
                    w_sb[:, :],
                    start=True, stop=True,
                ).then_inc(pe_fin, 1)

            for t in range(NT):
                if t >= 4:
                    pe.wait_ge(act_prog, t - 3)
                # one full-width start matmul opens the tile's accumulation
                # group and zeroes all 256 cols; every later matmul accumulates
                # (per-window starts would reset beyond their own columns).
                pe.matmul(
                    psb[t % 4][:, 0:4 * S],
                    zeros_f[:, 0:128],
                    zeros_f[:, 0:4 * S],
                    start=True, stop=False, skip_group_check=True,
                )
                for g in range(4):
                    pe.wait_ge(g_s[g][t % 4], 16 * (t // 4 + 1))
                    for b in range(TB):
                        gb = t * TB + b
                        if g == 0:
                            pe.wait_ge(dve_prog, gb + 1)
                        for q in range(RC):
                            i = g * RC + q
                            pe.matmul(
                                psb[t % 4][:, b * S:(b + 1) * S],
                                g_sb[:, t % 4, goff[t][g] + b * RC + q, :],
                                b_sb[:, gb % 8, i * S:(i + 1) * S],
                                start=False, stop=False,
                                skip_group_check=True,
                            )
                # overflow matmuls into the full 256-col tile window
                pe.wait_ge(dve_ovf, t + 1)
                nob = OBT[t]
                done = 0
                ins = None
                for g in range(4):
                    for ob in range(int(OB[t, g])):
                        ko = oboff[t][g] + ob
                        done += 1
                        ins = pe.matmul(
                            psb[t % 4][:, 0:4 * S],
                            g_sb[:, t % 4, goff[t][g] + 4 * RC + ob, :],
                            bo_sb[:, t % 4, ko * 4 * S:(ko + 1) * 4 * S],
                            start=False, stop=(done == nob),
                            skip_group_check=True,
                        )
                assert ins is not None, "tile with no overflow needs stop fix"
                ins.then_inc(pe_tiles, 1)
                if t >= 1:
                    for f in range(FPT * (t - 1), FPT * t):
                        final_mm(f)
            for f in range(FPT * (NT - 1), NF):
                final_mm(f)

        @block.scalar
        def _(act: bass.BassEngine):
            for t in range(NT):
                act.wait_ge(pe_tiles, t + 1)
                act.copy(agg_sb[:, t * TB * S:(t + 1) * TB * S],
                         psb[t % 4][:, :TB * S]).then_inc(act_prog, 1)
                if t >= 1:
                    for f in range(FPT * (t - 1), FPT * t):
                        act.wait_ge(dve_fin, f + 1)
                        act.dma_start(out[f * 128:(f + 1) * 128, :],
                                      ostage[:, f % 2, :]).then_inc(
                            ost_s[f % 2], 16)
            for f in range(FPT * (NT - 1), NF):
                act.wait_ge(dve_fin, f + 1)
                act.dma_start(out[f * 128:(f + 1) * 128, :],
                              ostage[:, f % 2, :]).then_inc(ost_s[f % 2], 16)

    nc.compile()
    return nc


def reassemble(results, cfg: Cfg):
    outs = [results[k]["out"] for k in range(cfg.n_cores)]
    full = np.concatenate(outs, axis=0)
    return full[: cfg.n_nodes]


_NC_CACHE = {}


def kernel(inputs, edge_row, edge_col, edge_weight, weight, bias):
    """Full GCN conv on 8 TRN2 cores; returns [100000, 128] float32."""
    import numpy as np
    from concourse.bass_utils import run_bass_kernel_spmd

    inputs = np.asarray(inputs, np.float32)
    edge_weight = np.asarray(edge_weight, np.float32)
    weight = np.asarray(weight, np.float32)
    bias = np.asarray(bias, np.float32)

    cfg = Cfg()
    cfg.OB = compute_budgets(edge_row, edge_col, cfg)
    in_maps = host_prep(inputs, edge_row, edge_col, edge_weight, cfg)
    add_consts(in_maps, weight, bias, cfg)
    key = cfg.OB.tobytes()
    if key not in _NC_CACHE:
        _NC_CACHE[key] = build(cfg)
    nc = _NC_CACHE[key]
    res = run_bass_kernel_spmd(nc, in_maps, core_ids=list(range(cfg.n_cores)))
    return reassemble(res.results, cfg).astype(np.float32)
